# revision 41
# baseline (speedup 1.0000x reference)
"""Involution kernel for Trainium2, 8 NeuronCores.

Sharding: data-parallel over (batch=4) x (H halves=2) -> 8 shards of
28 output rows each, with a 3-row halo (K=7 unfold). Host zero-pads the
image to 62 cols so every shard is a uniform [256, 36, 62] slab in a
flat "padded raster" layout (q = r*62 + w), stored fp16 twice (even and
odd-shifted copies) so every tap's x view is 4B-aligned for DVE 2x mode.

Per-core pipeline, channel-major, pixels PACKED to 28x56=1568 (padded
columns are never computed on; x reads use 3D row-strided APs). Work is
split into 4 quadrants (2 pixel halves x 2 channel tiles) of 49 tap
units each:
  A. reduce conv (1x1, BN+ReLU folded on host) -> r [128, 1568] fp16
  B. per unit: span conv emits per-channel dynamic weights via a
     host-replicated [128,128] fp16 weight block (one 2-bank PSUM tile,
     2 matmuls of 392 cols). The tap product (wm + b2) * x_shift is
     formed by one of two balanced paths:
       - A-path (~57%): ScalarE extracts wm+b2 to SBUF fp16, VectorE
         multiplies in fp16 at DVE 2x rate
       - P-path (~43%): Pool/GpSimd scalar_tensor_tensor reads the PSUM
         tile directly, fusing bias add + multiply
     Tap accumulation alternates between PE (identity matmul into a
     2-bank PSUM accumulator, emitted 3 units late so the in-order PE
     queue never head-of-line blocks) and VectorE (fp16 adds into an
     SBUF accumulator).
  C. per quadrant: identity matmuls merge the SBUF accumulator into
     PSUM, ScalarE copies out with +49*eps folded into the bias, DMA.
"""

import sys
import numpy as np

for _p in ("/opt/trn_rl_repo",):
    if _p not in sys.path:
        sys.path.insert(0, _p)

import concourse.bass as bass
import concourse.tile as tile
from concourse import mybir
from concourse.bass_utils import run_bass_kernel_spmd
import bass_rust

F32 = mybir.dt.float32
F16 = mybir.dt.float16

N_CORES = 8
C = 256
RED = 128
K = 7
K2 = 49
GC = 16
HW = 56
WPAD = 62            # padded width
NROW = 36            # rows in the padded x slab (1 pad + 34 shard + 1 pad)
XLEN = NROW * WPAD   # 2232
P = 28 * HW          # 1568 packed own pixels
HP = P // 2          # 784 per pixel half (14 rows)
CHK = HP // 2        # 392, PSUM chunk (fits one 2KB bank)
EPS49 = float(K2 * np.finfo(np.float32).eps)
ACC_DELAY = 6        # units of software pipelining for PE accumulation
ADD_DELAY = 3        # units of software pipelining for DVE adds of Pool prods


def _split_multi_waits(nc, maxw=1):
    """This walrus build caps sync-wait commands per instruction; move
    excess waits onto same-engine nops inserted immediately before."""
    ctr = 0
    for fn in nc.m.functions:
        for bb in fn.blocks:
            insts = bb.instructions  # live list
            i = 0
            while i < len(insts):
                ins = insts[i]
                si = ins.sync_info
                waits = list(si.on_wait) if si is not None else []
                if len(waits) > maxw:
                    excess, keep = waits[:-maxw], waits[-maxw:]
                    for j in range(0, len(excess), maxw):
                        ctr += 1
                        nop = mybir.InstNoOp(
                            name=f"waitsplit-{ctr}",
                            engine=ins.engine,
                            bass_nofuse=True,
                            sync_info=mybir.SyncInfo(
                                on_wait=excess[j:j + maxw], on_update=[]
                            ),
                        )
                        insts.insert(i, nop)
                        i += 1
                    ins.sync_info = bass_rust.SyncInfo(
                        on_wait=keep, on_update=list(si.on_update)
                    )
                i += 1


def build_program():
    nc = bass.Bass("TRN2", target_bir_lowering=False, num_devices=N_CORES)

    xhe_d = nc.dram_tensor("xhe", [C, XLEN], F16, kind="ExternalInput")
    xho_d = nc.dram_tensor("xho", [C, XLEN], F16, kind="ExternalInput")
    w1_d = nc.dram_tensor("w1L", [C, RED], F16, kind="ExternalInput")
    b1_d = nc.dram_tensor("b1L", [RED, 1], F32, kind="ExternalInput")
    w2_d = nc.dram_tensor("w2L", [RED, K2 * 2 * 128], F16, kind="ExternalInput")
    b2_d = nc.dram_tensor("b2L", [128, K2 * 2], F32, kind="ExternalInput")
    id_d = nc.dram_tensor("ident", [128, 128], F16, kind="ExternalInput")
    y_d = nc.dram_tensor("y", [C, 28, HW], F32, kind="ExternalOutput")

    # per-quadrant unit schedule, 49 tap units each. GPSIMD cannot touch
    # PSUM on real silicon, so the product paths are:
    #   P (~51%): Act extracts wm+b2 to SBUF fp16, Pool multiplies (STT)
    #   D (~33%): DVE scalar_tensor_tensor straight from PSUM (bias fused)
    #   A (~16%): Act extracts, DVE multiplies in fp16 at 2x
    # Accumulation: PE identity matmuls for P-units and half the D-units,
    # DVE fp16 adds for the rest.
    # strict interleave: D (Act-free, DVE STT from PSUM) every 3rd slot;
    # the rest mostly A (Act extract + DVE mult), ~5/49 P (Act extract +
    # Pool TT mult). Accumulation: ~36/49 on PE (identity matmul into
    # PSUM), ~13/49 on Pool (TT adds into an SBUF fp16 accumulator --
    # a pure sink, so Pool lateness never stalls anyone).
    paths = []
    ia = 0
    for u in range(K2):
        if u % 3 == 2 or u == K2 - 1:
            paths.append('D')
        else:
            ia += 1
            paths.append('P' if ia % 7 == 4 else 'A')
    accums = ['POOL' if (u * 13) % K2 < 13 else 'PE' for u in range(K2)]

    # w2 chunk tap-groups, in consumption order (t-major, host pre-reordered)
    KGRP = [(0, 13), (13, 13), (26, 13), (39, 10)]

    with tile.TileContext(nc) as tc:
        with (
            tc.tile_pool(name="sb", bufs=1) as sb,
            tc.tile_pool(name="wmp", bufs=8) as wmp,
            tc.tile_pool(name="prp", bufs=10) as prp,
            tc.tile_pool(name="osp", bufs=2) as osp,
            tc.tile_pool(name="ps", bufs=3, space="PSUM") as ps,
            tc.tile_pool(name="psacc", bufs=1, space="PSUM") as psacc,
        ):
            # DMA issue order is consumption order. Phase A reads xhe (PE has
            # no alignment constraint), and needs only slab rows 4..17 for the
            # first pixel half, so x tiles stream in two row chunks (rows
            # 0..21 cover every h=0 view; rows 22..35 the rest). xho (needed
            # by odd-tap consumers from ~unit 1) trails the first w2 chunk.
            xhe_t = []
            xho_t = []
            for ci in range(2):
                the = sb.tile([128, XLEN], F16, tag=f"xhe_{ci}")
                nc.sync.dma_start(out=the[:], in_=xhe_d[ci * 128:(ci + 1) * 128, :])
                xhe_t.append(the)
            w1sb = []
            for ci in range(2):
                w = sb.tile([128, RED], F16, tag=f"w1_{ci}")
                nc.sync.dma_start(out=w[:], in_=w1_d[ci * 128:(ci + 1) * 128, :])
                w1sb.append(w)
            b1sb = sb.tile([RED, 1], F32, tag="b1")
            nc.sync.dma_start(out=b1sb[:], in_=b1_d[:, :])
            b2sb = sb.tile([128, K2 * 2], F32, tag="b2")
            nc.sync.dma_start(out=b2sb[:], in_=b2_d[:, :])
            idsb = sb.tile([128, 128], F16, tag="ident")
            nc.sync.dma_start(out=idsb[:], in_=id_d[:, :])

            w2sb = {}        # (t, group) -> tile
            def load_w2(t, gi, eng=None):
                k0, klen = KGRP[gi]
                w = sb.tile([RED, klen * 128], F16, tag=f"w2_{t}_{gi}")
                c0 = (t * K2 + k0) * 128
                (eng or nc.sync).dma_start(out=w[:], in_=w2_d[:, c0:c0 + klen * 128])
                w2sb[(t, gi)] = w

            load_w2(0, 0)
            for ci in range(2):
                tho = sb.tile([128, XLEN], F16, tag=f"xho_{ci}")
                nc.sync.dma_start(out=tho[:],
                                  in_=xho_d[ci * 128:(ci + 1) * 128, :])
                xho_t.append(tho)
                if ci == 0:
                    load_w2(0, 1)
            for gi in range(2, 4):
                load_w2(0, gi)
            for gi in range(4):
                load_w2(1, gi)

            xhe = [t[:].rearrange("p (r w) -> p r w", w=WPAD) for t in xhe_t]
            xho = [t[:].rearrange("p (r w) -> p r w", w=WPAD) for t in xho_t]

            def w2blk(t, k):
                for gi, (k0, klen) in enumerate(KGRP):
                    if k0 <= k < k0 + klen:
                        return w2sb[(t, gi)][:, (k - k0) * 128:(k - k0 + 1) * 128]
                raise AssertionError

            def xview(t, k, h):
                """x operand view [128, 14, 56] for channel tile t, tap k,
                pixel half h. Always starts at an even flat element so fp16
                rows are 4B-aligned (xho holds the odd-shifted copy)."""
                di, dj = k // K, k % K
                r0 = di + 1 + 14 * h
                if dj % 2 == 0:          # flat base 62*(di+1+14h)+dj: even dj
                    return xhe[t][:, r0:r0 + 14, dj:dj + HW]
                return xho[t][:, r0:r0 + 14, dj - 1:dj - 1 + HW]

            # Phase A: r = relu(w1' @ x + b1')  [128, P] fp16, packed pixels.
            # Center-tap (di=dj=3) view read straight from xhe (PE moving
            # operands have no alignment constraint). h=0 runs up front (its
            # x rows arrive first); h=1 is injected a few units into the
            # first quadrant so it never delays the phase B start. Relus are
            # chunked so the first span can fire off chunk 0 alone.
            r_sb = sb.tile([RED, P], F16, tag="r")

            def emit_phase_a(h):
                rps = ps.tile([128, 2, 512], F32, tag="wm", name=f"rps_{h}")
                for ci in range(2):
                    for cc in range(2):
                        r0 = 4 + 14 * h + 7 * cc
                        nc.tensor.matmul(
                            rps[:, cc, 0:CHK],
                            w1sb[ci][:],
                            xhe[ci][:, r0:r0 + 7, 3:3 + HW],
                            start=(ci == 0),
                            stop=(ci == 1),
                        )
                nc.scalar.activation(
                    out=r_sb[:, h * HP:(h + 1) * HP],
                    in_=rps[:, :, 0:CHK],
                    func=mybir.ActivationFunctionType.Relu,
                    bias=b1sb[:, 0:1],
                    scale=1.0,
                )

            emit_phase_a(0)
            emit_phase_a(1)

            # Phase B: 4 quadrants of 49 tap units. The phase-C copy+DMA of
            # each quadrant is deferred a few units into the next one so the
            # Act queue is never head-of-line blocked at the boundary.
            deferred_c = [None]

            def flush_phase_c():
                if deferred_c[0] is not None:
                    deferred_c[0]()
                    deferred_c[0] = None

            for h in range(2):
                for t in range(2):
                    acc = psacc.tile([128, 2, 512], F32, tag="acc",
                                     name=f"acc_{h}_{t}")
                    acc_sb = None
                    first_pe = True
                    first_pool = True
                    pending = []   # (dest_tile, was_first_pe) awaiting PE accum
                    padds = []     # prod tiles awaiting a Pool TT add
                    for u in range(K2):
                        if u == 3:
                            flush_phase_c()
                        k = u
                        idx = t * K2 + k
                        wmps = ps.tile([128, 2, 512], F32, tag="wm")
                        for cc in range(2):
                            nc.tensor.matmul(
                                wmps[:, cc, 0:CHK],
                                w2blk(t, k),
                                r_sb[:, h * HP + cc * CHK:
                                     h * HP + (cc + 1) * CHK],
                                start=True,
                                stop=True,
                            )
                        # software-pipelined PE accumulation
                        while len(pending) > ACC_DELAY - 1:
                            dst, fp = pending.pop(0)
                            for cc in range(2):
                                nc.tensor.matmul(
                                    acc[:, cc, 0:CHK],
                                    idsb[:],
                                    dst[:, cc * CHK:(cc + 1) * CHK],
                                    start=fp,
                                    stop=False,
                                    skip_group_check=True,
                                )
                        xv = xview(t, k, h)

                        if accums[u] == 'POOL' and first_pool:
                            acc_sb = sb.tile([128, HP], F16, tag=f"asb_{h}_{t}")
                            dest = acc_sb
                        else:
                            dest = prp.tile([128, HP], F16, tag="prod")
                        dv = dest[:].rearrange("p (r w) -> p r w", w=HW)

                        if paths[u] == 'D':
                            # DVE straight from PSUM, bias fused (legal on
                            # HW for DVE, unlike GPSIMD)
                            nc.vector.scalar_tensor_tensor(
                                out=dv[:, :, :],
                                in0=wmps[:, :, 0:CHK],
                                scalar=b2sb[:, idx:idx + 1],
                                in1=xv,
                                op0=mybir.AluOpType.add,
                                op1=mybir.AluOpType.mult,
                            )
                        else:
                            wmb = wmp.tile([128, HP], F16, tag="wmb")
                            nc.scalar.activation(
                                out=wmb[:],
                                in_=wmps[:, :, 0:CHK],
                                func=mybir.ActivationFunctionType.Identity,
                                bias=b2sb[:, idx:idx + 1],
                                scale=1.0,
                            )
                            if paths[u] == 'A':
                                nc.vector.tensor_mul(
                                    dv[:, :, :],
                                    wmb[:].rearrange("p (r w) -> p r w", w=HW),
                                    xv,
                                )
                            else:
                                nc.gpsimd.tensor_tensor(
                                    dv[:, :, :],
                                    wmb[:].rearrange("p (r w) -> p r w", w=HW),
                                    xv,
                                    op=mybir.AluOpType.mult,
                                )

                        if accums[u] == 'PE':
                            pending.append((dest, first_pe))
                            first_pe = False
                        elif first_pool:
                            first_pool = False
                        else:
                            # Pool accumulates into SBUF fp16, a few units
                            # late so the prod is certainly ready
                            padds.append((u, dest))
                        while padds and padds[0][0] <= u - ADD_DELAY:
                            _, d = padds.pop(0)
                            nc.gpsimd.tensor_tensor(
                                acc_sb[:], acc_sb[:], d[:],
                                op=mybir.AluOpType.add,
                            )

                    for _, d in padds:
                        nc.gpsimd.tensor_tensor(
                            acc_sb[:], acc_sb[:], d[:],
                            op=mybir.AluOpType.add,
                        )
                    for dst, fp in pending:
                        for cc in range(2):
                            nc.tensor.matmul(
                                acc[:, cc, 0:CHK],
                                idsb[:],
                                dst[:, cc * CHK:(cc + 1) * CHK],
                                start=fp,
                                stop=False,
                                skip_group_check=True,
                            )
                    # merge SBUF accumulator; copy out + DMA are deferred
                    # into the next quadrant
                    for cc in range(2):
                        nc.tensor.matmul(
                            acc[:, cc, 0:CHK],
                            idsb[:],
                            acc_sb[:, cc * CHK:(cc + 1) * CHK],
                            start=False,
                            stop=True,
                            skip_group_check=True,
                        )

                    def make_phase_c(acc=acc, h=h, t=t):
                        def emit():
                            osb = osp.tile([128, HP], F32, tag="osb",
                                           name=f"osb_{h}_{t}")
                            nc.scalar.activation(
                                out=osb[:],
                                in_=acc[:, :, 0:CHK],
                                func=mybir.ActivationFunctionType.Copy,
                                bias=EPS49,
                                scale=1.0,
                            )
                            osbv = osb[:].rearrange("p (r w) -> p r w", w=HW)
                            nc.sync.dma_start(
                                out=y_d[t * 128:(t + 1) * 128,
                                        14 * h:14 * (h + 1), :],
                                in_=osbv[:, :, :],
                            )
                        return emit

                    deferred_c[0] = make_phase_c()
            flush_phase_c()
    _split_multi_waits(nc)
    return nc


_PROGRAM = None
LAST_RESULT = None


def kernel(x, w1, b1, gamma, beta, run_mean, run_var, w2, b2):
    global _PROGRAM, LAST_RESULT
    x = np.asarray(x, dtype=np.float32)
    w1 = np.asarray(w1, dtype=np.float32)
    b1 = np.asarray(b1, dtype=np.float32)
    gamma = np.asarray(gamma, dtype=np.float32)
    beta = np.asarray(beta, dtype=np.float32)
    run_mean = np.asarray(run_mean, dtype=np.float32)
    run_var = np.asarray(run_var, dtype=np.float32)
    w2 = np.asarray(w2, dtype=np.float32)
    b2 = np.asarray(b2, dtype=np.float32)

    B = x.shape[0]
    # fold BN (eval) into the 1x1 reduce conv
    s = gamma / np.sqrt(run_var + 1e-5)
    w1p = w1 * s[:, None]
    b1p = (b1 - run_mean) * s + beta

    w1L = np.ascontiguousarray(w1p.T).astype(np.float16)  # [C, RED]
    b1L = np.ascontiguousarray(b1p[:, None])              # [RED, 1]

    # replicate w2 rows so the span matmul emits per-channel weights:
    # lhsT block for (channel tile t, tap k) at idx2 = t*49+k: [RED, 128],
    # col c' uses w2 row (c'//16 + 8t)*49 + k (t-major so the device can
    # stream chunks in consumption order)
    cloc = np.arange(128)
    g8 = cloc // GC
    w2L = np.empty((RED, K2 * 2 * 128), dtype=np.float16)
    b2L = np.empty((128, K2 * 2), dtype=np.float32)
    w2T = w2.T.astype(np.float16)  # [RED, G*K2]
    for k in range(K2):
        for t in range(2):
            rows = (g8 + 8 * t) * K2 + k
            idx2 = t * K2 + k
            w2L[:, idx2 * 128:(idx2 + 1) * 128] = w2T[:, rows]
            b2L[:, idx2] = b2[rows]
    ident = np.eye(128, dtype=np.float16)

    xpad = np.zeros((B, C, WPAD, WPAD), dtype=np.float16)
    xpad[:, :, 3:3 + HW, 3:3 + HW] = x

    in_maps = []
    for core in range(N_CORES):
        b, half = core // 2, core % 2
        xhe = np.zeros((C, NROW, WPAD), dtype=np.float16)
        xhe[:, 1:35, :] = xpad[b, :, half * 28: half * 28 + 34, :]
        xhe = xhe.reshape(C, XLEN)
        xho = np.zeros_like(xhe)
        xho[:, :XLEN - 1] = xhe[:, 1:]
        in_maps.append({
            "xhe": xhe, "xho": xho,
            "w1L": w1L, "b1L": b1L, "w2L": w2L, "b2L": b2L,
            "ident": ident,
        })

    if _PROGRAM is None:
        _PROGRAM = build_program()
    res = run_bass_kernel_spmd(_PROGRAM, in_maps, list(range(N_CORES)))
    LAST_RESULT = res

    y = np.empty((B, C, HW, HW), dtype=np.float32)
    for core in range(N_CORES):
        b, half = core // 2, core % 2
        y[b, :, half * 28:(half + 1) * 28, :] = res.results[core]["y"]
    return y


# revision 66
# speedup vs baseline: 1.1794x; 1.1794x over previous
"""Involution kernel for Trainium2, 8 NeuronCores.

Sharding: data-parallel over (batch=4) x (H halves=2) -> 8 shards of
28 output rows each, with a 3-row halo (K=7 unfold). Host zero-pads the
image to 62 cols so every shard is a uniform [256, 36, 62] slab in a
flat "padded raster" layout (q = r*62 + w), stored fp16 twice (even and
odd-shifted copies) so every tap's x view is 4B-aligned for DVE 2x mode.

Per-core pipeline, channel-major, pixels PACKED to 28x56=1568 (padded
columns are never computed on; x reads use 3D row-strided APs). Work is
split into 4 quadrants (2 pixel halves x 2 channel tiles) of 49 tap
units each:
  A. reduce conv (1x1, BN+ReLU folded on host) -> r [128, 1568] fp16
  B. per unit: span conv emits per-channel dynamic weights via a
     host-replicated [128,128] fp16 weight block (one 2-bank PSUM tile,
     2 matmuls of 392 cols). The tap product (wm + b2) * x_shift is
     formed by one of three engine paths (GPSIMD cannot touch PSUM on
     real silicon, so every PSUM read is ScalarE/VectorE):
       - A (~27/49): ScalarE extracts wm+b2 to SBUF fp16, VectorE
         multiplies in fp16 at DVE 2x rate, emitted 2 units late so the
         in-order DVE queue never head-of-line blocks on ScalarE
       - D (~17/49, every 3rd slot): VectorE scalar_tensor_tensor
         straight from PSUM, fusing bias add + multiply
       - P (~3/49): ScalarE extracts, Pool tensor_tensor multiplies
     Tap accumulation: ~40/49 on PE (identity matmul into a 2-bank PSUM
     accumulator, software-pipelined several units behind the products)
     and ~9/49 on Pool (fp16 tensor_tensor adds into an SBUF
     accumulator -- a pure sink nothing downstream waits on).
  C. per quadrant: identity matmuls merge the SBUF accumulator into
     PSUM, ScalarE copies out with +49*eps folded into the bias, DMA;
     copy+DMA are deferred into the next quadrant so the ScalarE queue
     is never blocked at the boundary.
"""

import sys
import numpy as np

for _p in ("/opt/trn_rl_repo",):
    if _p not in sys.path:
        sys.path.insert(0, _p)

import concourse.bass as bass
import concourse.tile as tile
from concourse import mybir
from concourse.bass_utils import run_bass_kernel_spmd
import bass_rust

F32 = mybir.dt.float32
F16 = mybir.dt.float16

N_CORES = 8
C = 256
RED = 128
K = 7
K2 = 49
GC = 16
HW = 56
WPAD = 62            # padded width
NROW = 36            # rows in the padded x slab (1 pad + 34 shard + 1 pad)
XLEN = NROW * WPAD   # 2232
P = 28 * HW          # 1568 packed own pixels
HP = P // 2          # 784 per pixel half (14 rows)
CHK = HP // 2        # 392, PSUM chunk (fits one 2KB bank)
EPS49 = float(K2 * np.finfo(np.float32).eps)
ACC_DELAY = 6        # units of software pipelining for PE accumulation
ADD_DELAY = 3        # units of software pipelining for DVE adds of Pool prods


def _split_multi_waits(nc, maxw=1):
    """This walrus build caps sync-wait commands per instruction; move
    excess waits onto same-engine nops inserted immediately before."""
    ctr = 0
    for fn in nc.m.functions:
        for bb in fn.blocks:
            insts = bb.instructions  # live list
            i = 0
            while i < len(insts):
                ins = insts[i]
                si = ins.sync_info
                waits = list(si.on_wait) if si is not None else []
                if len(waits) > maxw:
                    excess, keep = waits[:-maxw], waits[-maxw:]
                    for j in range(0, len(excess), maxw):
                        ctr += 1
                        nop = mybir.InstNoOp(
                            name=f"waitsplit-{ctr}",
                            engine=ins.engine,
                            bass_nofuse=True,
                            sync_info=mybir.SyncInfo(
                                on_wait=excess[j:j + maxw], on_update=[]
                            ),
                        )
                        insts.insert(i, nop)
                        i += 1
                    ins.sync_info = bass_rust.SyncInfo(
                        on_wait=keep, on_update=list(si.on_update)
                    )
                i += 1


def build_program():
    nc = bass.Bass("TRN2", target_bir_lowering=False, num_devices=N_CORES)

    xhe_d = nc.dram_tensor("xhe", [C, XLEN], F16, kind="ExternalInput")
    xho_d = nc.dram_tensor("xho", [C, XLEN], F16, kind="ExternalInput")
    w1_d = nc.dram_tensor("w1L", [C, RED], F16, kind="ExternalInput")
    b1_d = nc.dram_tensor("b1L", [RED, 1], F32, kind="ExternalInput")
    w2_d = nc.dram_tensor("w2L", [RED, K2 * 2 * 128], F16, kind="ExternalInput")
    b2_d = nc.dram_tensor("b2L", [128, K2 * 2], F32, kind="ExternalInput")
    id_d = nc.dram_tensor("ident", [128, 128], F16, kind="ExternalInput")
    y_d = nc.dram_tensor("y", [C, 28, HW], F32, kind="ExternalOutput")

    # per-quadrant unit schedule (counts tuned empirically on the
    # TimelineSim model): D every 3rd slot keeps the Act queue from
    # bunching; the Pool accum count/phase sits at a measured optimum.
    paths = []
    ia = 0
    for u in range(K2):
        if u % 5 in (1, 3) or u == K2 - 1:
            paths.append('D')
        else:
            ia += 1
            paths.append('P' if ia % 11 == 5 else 'A')
    accums = ['POOL' if (u * 9) % K2 < 9 else 'PE' for u in range(K2)]

    # w2 chunk tap-groups, in consumption order (t-major, host pre-reordered)
    KGRP = [(0, 13), (13, 13), (26, 13), (39, 10)]

    with tile.TileContext(nc) as tc:
        with (
            tc.tile_pool(name="sb", bufs=1) as sb,
            tc.tile_pool(name="wmp", bufs=6) as wmp,
            tc.tile_pool(name="prp", bufs=10) as prp,
            tc.tile_pool(name="osp", bufs=2) as osp,
            tc.tile_pool(name="ps", bufs=3, space="PSUM") as ps,
            tc.tile_pool(name="psacc", bufs=1, space="PSUM") as psacc,
        ):
            # DMA issue order is consumption order. Phase A reads xhe (PE has
            # no alignment constraint), and needs only slab rows 4..17 for the
            # first pixel half, so x tiles stream in two row chunks (rows
            # 0..21 cover every h=0 view; rows 22..35 the rest). xho (needed
            # by odd-tap consumers from ~unit 1) trails the first w2 chunk.
            xhe_t = []
            xho_t = []
            for ci in range(2):
                the = sb.tile([128, XLEN], F16, tag=f"xhe_{ci}")
                eng = nc.sync if ci == 0 else nc.scalar
                eng.dma_start(out=the[:], in_=xhe_d[ci * 128:(ci + 1) * 128, :])
                xhe_t.append(the)
            w1sb = []
            for ci in range(2):
                w = sb.tile([128, RED], F16, tag=f"w1_{ci}")
                eng = nc.sync if ci == 0 else nc.scalar
                eng.dma_start(out=w[:], in_=w1_d[ci * 128:(ci + 1) * 128, :])
                w1sb.append(w)
            b1sb = sb.tile([RED, 1], F32, tag="b1")
            nc.sync.dma_start(out=b1sb[:], in_=b1_d[:, :])
            b2sb = sb.tile([128, K2 * 2], F32, tag="b2")
            nc.scalar.dma_start(out=b2sb[:], in_=b2_d[:, :])
            idsb = sb.tile([128, 128], F16, tag="ident")
            nc.scalar.dma_start(out=idsb[:], in_=id_d[:, :])

            w2sb = {}        # (t, group) -> tile
            def load_w2(t, gi, eng=None):
                k0, klen = KGRP[gi]
                w = sb.tile([RED, klen * 128], F16, tag=f"w2_{t}_{gi}")
                c0 = (t * K2 + k0) * 128
                (eng or nc.sync).dma_start(out=w[:], in_=w2_d[:, c0:c0 + klen * 128])
                w2sb[(t, gi)] = w

            load_w2(0, 0)
            for ci in range(2):
                tho = sb.tile([128, XLEN], F16, tag=f"xho_{ci}")
                eng = nc.sync if ci == 0 else nc.scalar
                eng.dma_start(out=tho[:],
                              in_=xho_d[ci * 128:(ci + 1) * 128, :])
                xho_t.append(tho)
                if ci == 0:
                    load_w2(0, 1)
            for gi in range(2, 4):
                load_w2(0, gi)
            for gi in range(4):
                load_w2(1, gi)

            xhe = [t[:].rearrange("p (r w) -> p r w", w=WPAD) for t in xhe_t]
            xho = [t[:].rearrange("p (r w) -> p r w", w=WPAD) for t in xho_t]

            def w2blk(t, k):
                for gi, (k0, klen) in enumerate(KGRP):
                    if k0 <= k < k0 + klen:
                        return w2sb[(t, gi)][:, (k - k0) * 128:(k - k0 + 1) * 128]
                raise AssertionError

            def xview(t, k, h):
                """x operand view [128, 14, 56] for channel tile t, tap k,
                pixel half h. Always starts at an even flat element so fp16
                rows are 4B-aligned (xho holds the odd-shifted copy)."""
                di, dj = k // K, k % K
                r0 = di + 1 + 14 * h
                if dj % 2 == 0:          # flat base 62*(di+1+14h)+dj: even dj
                    return xhe[t][:, r0:r0 + 14, dj:dj + HW]
                return xho[t][:, r0:r0 + 14, dj - 1:dj - 1 + HW]

            # Phase A: r = relu(w1' @ x + b1')  [128, P] fp16, packed pixels.
            # Center-tap (di=dj=3) view read straight from xhe (PE moving
            # operands have no alignment constraint). h=0 runs up front (its
            # x rows arrive first); h=1 is injected a few units into the
            # first quadrant so it never delays the phase B start. Relus are
            # chunked so the first span can fire off chunk 0 alone.
            r_sb = sb.tile([RED, P], F16, tag="r")

            def emit_phase_a(h):
                rps = ps.tile([128, 2, 512], F32, tag="wm", name=f"rps_{h}")
                for ci in range(2):
                    for cc in range(2):
                        r0 = 4 + 14 * h + 7 * cc
                        nc.tensor.matmul(
                            rps[:, cc, 0:CHK],
                            w1sb[ci][:],
                            xhe[ci][:, r0:r0 + 7, 3:3 + HW],
                            start=(ci == 0),
                            stop=(ci == 1),
                        )
                nc.scalar.activation(
                    out=r_sb[:, h * HP:(h + 1) * HP],
                    in_=rps[:, :, 0:CHK],
                    func=mybir.ActivationFunctionType.Relu,
                    bias=b1sb[:, 0:1],
                    scale=1.0,
                )

            emit_phase_a(0)
            emit_phase_a(1)

            # Phase B: 4 quadrants of 49 tap units. The phase-C copy+DMA of
            # each quadrant is deferred a few units into the next one so the
            # Act queue is never head-of-line blocked at the boundary.
            deferred_c = [None]

            def flush_phase_c():
                if deferred_c[0] is not None:
                    deferred_c[0]()
                    deferred_c[0] = None

            for h in range(2):
                for t in range(2):
                    acc = psacc.tile([128, 2, 512], F32, tag="acc",
                                     name=f"acc_{h}_{t}")
                    acc_sb = None
                    first_pe = True
                    first_pool = True
                    pending = []   # (dest_tile, was_first_pe) awaiting PE accum
                    padds = []     # prod tiles awaiting a Pool TT add
                    dmults = []    # deferred A-unit DVE mults (1 unit late so
                                   # the in-order DVE queue never waits on Act)

                    def flush_dmults(upto):
                        while dmults and dmults[0][0] <= upto:
                            _, wmb_, dv_, xv_ = dmults.pop(0)
                            nc.vector.tensor_mul(
                                dv_[:, :, :],
                                wmb_[:].rearrange("p (r w) -> p r w", w=HW),
                                xv_,
                            )
                    for u in range(K2):
                        if u == 3:
                            flush_phase_c()
                        k = u
                        idx = t * K2 + k
                        wmps = ps.tile([128, 2, 512], F32, tag="wm")
                        for cc in range(2):
                            nc.tensor.matmul(
                                wmps[:, cc, 0:CHK],
                                w2blk(t, k),
                                r_sb[:, h * HP + cc * CHK:
                                     h * HP + (cc + 1) * CHK],
                                start=True,
                                stop=True,
                            )
                        flush_dmults(u - 2)
                        # software-pipelined PE accumulation; the window must
                        # stay wider than the deferred-mult delay or an accum
                        # could be emitted before its product
                        while len(pending) > ACC_DELAY - 1:
                            dst, fp = pending.pop(0)
                            for cc in range(2):
                                nc.tensor.matmul(
                                    acc[:, cc, 0:CHK],
                                    idsb[:],
                                    dst[:, cc * CHK:(cc + 1) * CHK],
                                    start=fp,
                                    stop=False,
                                    skip_group_check=True,
                                )
                        xv = xview(t, k, h)

                        if accums[u] == 'POOL' and first_pool:
                            acc_sb = sb.tile([128, HP], F16, tag=f"asb_{h}_{t}")
                            dest = acc_sb
                        else:
                            dest = prp.tile([128, HP], F16, tag="prod")
                        dv = dest[:].rearrange("p (r w) -> p r w", w=HW)

                        if paths[u] == 'D':
                            # DVE straight from PSUM, bias fused (legal on
                            # HW for DVE, unlike GPSIMD)
                            nc.vector.scalar_tensor_tensor(
                                out=dv[:, :, :],
                                in0=wmps[:, :, 0:CHK],
                                scalar=b2sb[:, idx:idx + 1],
                                in1=xv,
                                op0=mybir.AluOpType.add,
                                op1=mybir.AluOpType.mult,
                            )
                        else:
                            wmb = wmp.tile([128, HP], F16, tag="wmb")
                            nc.scalar.activation(
                                out=wmb[:],
                                in_=wmps[:, :, 0:CHK],
                                func=mybir.ActivationFunctionType.Identity,
                                bias=b2sb[:, idx:idx + 1],
                                scale=1.0,
                            )
                            if paths[u] == 'A':
                                dmults.append((u, wmb, dv, xv))
                            else:
                                nc.gpsimd.tensor_tensor(
                                    dv[:, :, :],
                                    wmb[:].rearrange("p (r w) -> p r w", w=HW),
                                    xv,
                                    op=mybir.AluOpType.mult,
                                )

                        if accums[u] == 'PE':
                            pending.append((dest, first_pe))
                            first_pe = False
                        elif first_pool:
                            first_pool = False
                        else:
                            # Pool accumulates into SBUF fp16, a few units
                            # late so the prod is certainly ready
                            padds.append((u, dest))
                        while padds and padds[0][0] <= u - ADD_DELAY:
                            _, d = padds.pop(0)
                            nc.gpsimd.tensor_tensor(
                                acc_sb[:], acc_sb[:], d[:],
                                op=mybir.AluOpType.add,
                            )

                    flush_dmults(K2)
                    for _, d in padds:
                        nc.gpsimd.tensor_tensor(
                            acc_sb[:], acc_sb[:], d[:],
                            op=mybir.AluOpType.add,
                        )
                    for dst, fp in pending:
                        for cc in range(2):
                            nc.tensor.matmul(
                                acc[:, cc, 0:CHK],
                                idsb[:],
                                dst[:, cc * CHK:(cc + 1) * CHK],
                                start=fp,
                                stop=False,
                                skip_group_check=True,
                            )
                    # merge SBUF accumulator; copy out + DMA are deferred
                    # into the next quadrant
                    for cc in range(2):
                        nc.tensor.matmul(
                            acc[:, cc, 0:CHK],
                            idsb[:],
                            acc_sb[:, cc * CHK:(cc + 1) * CHK],
                            start=False,
                            stop=True,
                            skip_group_check=True,
                        )

                    def make_phase_c(acc=acc, h=h, t=t):
                        final = (h, t) == (1, 1)
                        def emit():
                            osb = osp.tile([128, HP], F32, tag="osb",
                                           name=f"osb_{h}_{t}")
                            osbv = osb[:].rearrange("p (r w) -> p r w", w=HW)
                            if final:
                                # chunked so the first half streams out while
                                # the second merge matmul is still running
                                for cc in range(2):
                                    nc.scalar.activation(
                                        out=osb[:, cc * CHK:(cc + 1) * CHK],
                                        in_=acc[:, cc, 0:CHK],
                                        func=mybir.ActivationFunctionType.Copy,
                                        bias=EPS49,
                                        scale=1.0,
                                    )
                                    nc.sync.dma_start(
                                        out=y_d[t * 128:(t + 1) * 128,
                                                14 * h + 7 * cc:
                                                14 * h + 7 * (cc + 1), :],
                                        in_=osbv[:, 7 * cc:7 * (cc + 1), :],
                                    )
                                return
                            nc.scalar.activation(
                                out=osb[:],
                                in_=acc[:, :, 0:CHK],
                                func=mybir.ActivationFunctionType.Copy,
                                bias=EPS49,
                                scale=1.0,
                            )
                            nc.sync.dma_start(
                                out=y_d[t * 128:(t + 1) * 128,
                                        14 * h:14 * (h + 1), :],
                                in_=osbv[:, :, :],
                            )
                        return emit

                    deferred_c[0] = make_phase_c()
            flush_phase_c()
    _split_multi_waits(nc)
    return nc


_PROGRAM = None
LAST_RESULT = None


def kernel(x, w1, b1, gamma, beta, run_mean, run_var, w2, b2):
    global _PROGRAM, LAST_RESULT
    x = np.asarray(x, dtype=np.float32)
    w1 = np.asarray(w1, dtype=np.float32)
    b1 = np.asarray(b1, dtype=np.float32)
    gamma = np.asarray(gamma, dtype=np.float32)
    beta = np.asarray(beta, dtype=np.float32)
    run_mean = np.asarray(run_mean, dtype=np.float32)
    run_var = np.asarray(run_var, dtype=np.float32)
    w2 = np.asarray(w2, dtype=np.float32)
    b2 = np.asarray(b2, dtype=np.float32)

    B = x.shape[0]
    # fold BN (eval) into the 1x1 reduce conv
    s = gamma / np.sqrt(run_var + 1e-5)
    w1p = w1 * s[:, None]
    b1p = (b1 - run_mean) * s + beta

    w1L = np.ascontiguousarray(w1p.T).astype(np.float16)  # [C, RED]
    b1L = np.ascontiguousarray(b1p[:, None])              # [RED, 1]

    # replicate w2 rows so the span matmul emits per-channel weights:
    # lhsT block for (channel tile t, tap k) at idx2 = t*49+k: [RED, 128],
    # col c' uses w2 row (c'//16 + 8t)*49 + k (t-major so the device can
    # stream chunks in consumption order)
    cloc = np.arange(128)
    g8 = cloc // GC
    w2L = np.empty((RED, K2 * 2 * 128), dtype=np.float16)
    b2L = np.empty((128, K2 * 2), dtype=np.float32)
    w2T = w2.T.astype(np.float16)  # [RED, G*K2]
    for k in range(K2):
        for t in range(2):
            rows = (g8 + 8 * t) * K2 + k
            idx2 = t * K2 + k
            w2L[:, idx2 * 128:(idx2 + 1) * 128] = w2T[:, rows]
            b2L[:, idx2] = b2[rows]
    ident = np.eye(128, dtype=np.float16)

    xpad = np.zeros((B, C, WPAD, WPAD), dtype=np.float16)
    xpad[:, :, 3:3 + HW, 3:3 + HW] = x

    in_maps = []
    for core in range(N_CORES):
        b, half = core // 2, core % 2
        xhe = np.zeros((C, NROW, WPAD), dtype=np.float16)
        xhe[:, 1:35, :] = xpad[b, :, half * 28: half * 28 + 34, :]
        xhe = xhe.reshape(C, XLEN)
        xho = np.zeros_like(xhe)
        xho[:, :XLEN - 1] = xhe[:, 1:]
        in_maps.append({
            "xhe": xhe, "xho": xho,
            "w1L": w1L, "b1L": b1L, "w2L": w2L, "b2L": b2L,
            "ident": ident,
        })

    if _PROGRAM is None:
        _PROGRAM = build_program()
    res = run_bass_kernel_spmd(_PROGRAM, in_maps, list(range(N_CORES)))
    LAST_RESULT = res

    y = np.empty((B, C, HW, HW), dtype=np.float32)
    for core in range(N_CORES):
        b, half = core // 2, core % 2
        y[b, :, half * 28:(half + 1) * 28, :] = res.results[core]["y"]
    return y
